# revision 1
# baseline (speedup 1.0000x reference)
"""Chamfer loss Trainium2 kernel (data-parallel over batch, 8 NeuronCores).

Problem: x, y (8, 4096, 3) fp32; loss = mean_n [ mean_w min_v ||x_nv - y_nw||
+ mean_v min_w ||x_nv - y_nw|| ] (scalar fp32).

Per core (one batch):
  - Host packs augmented operands AX, AY [24, 4096] bf16 via an
    error-compensated 3-way hi/mid/lo split (products hh, hm, mh, hl, lh,
    mm + 3-way-split norm rows) so the PE gram matmul produces
    sq[v,w] = ||x_v||^2 + ||y_w||^2 - 2 x_v.y_w to ~1e-7 absolute
    accuracy while streaming at bf16 rate (1 col/cycle).
  - PE: 32 m-blocks x 8 matmuls of [24,128]^T @ [24,512] -> PSUM
    [128, 2048] groups (4 banks, double buffered).
  - ACT (ScalarE): evacuates each PSUM group to SBUF fp16 with a fused
    relu clamp and x256 pre-scale (keeps tiny sq minima in fp16 normals).
  - DVE (critical path, ~158 us busy): row-direction min (min over w per
    v): fp16 2x-mode fold tree per m-block + one batched strided reduce
    per 4 m-blocks; col-direction min (min over v per w): one fp16
    running-min chain over a [128, 4096] accumulator per m-block.
  - Epilogue: PE-transposes of the col accumulator + strided reduces ->
    per-w mins; single ACT sqrt with fused free-dim sum -> stot[128, 1].
  - Host: sum the 128 partials per core, scale by 1/(V*sqrt(256)),
    average the 8 per-core losses.
"""

import sys

sys.path.insert(0, "/opt/trn_rl_repo")

from contextlib import ExitStack

import ml_dtypes
import numpy as np

import concourse.bacc as bacc
import concourse.tile as tile
from concourse import mybir
from concourse.bass_utils import run_bass_kernel_spmd

BF16 = ml_dtypes.bfloat16

P = 128
V = 4096
KA = 24  # augmented contraction dim (3-way hi/mid/lo split)
NMM = 512  # matmul moving free dim (one fp32 PSUM bank)
GRP = 1024  # PSUM group (2 banks)
NG = V // GRP  # 2 groups per m-block
MB = V // P  # 32 m-blocks
RB = 4  # m-blocks per batched row-min reduce
SCL = 256.0  # fp16 pre-scale: keeps tiny sq minima out of fp16 subnormals

_cache = {}


def _build_nc():
    F32 = mybir.dt.float32
    F16 = mybir.dt.float16
    mn = mybir.AluOpType.min
    X = mybir.AxisListType.X

    nc = bacc.Bacc("TRN2", target_bir_lowering=False)
    ax_d = nc.declare_dram_parameter("ax", [KA, V], mybir.dt.bfloat16, isOutput=False)
    ay_d = nc.declare_dram_parameter("ay", [KA, V], mybir.dt.bfloat16, isOutput=False)
    idh_d = nc.declare_dram_parameter("identh", [P, P], F16, isOutput=False)
    loss_d = nc.declare_dram_parameter("loss", [P, 1], F32, isOutput=True)

    with tile.TileContext(nc) as tc, ExitStack() as ctx:
        const = ctx.enter_context(tc.tile_pool(name="const", bufs=1))
        accs = ctx.enter_context(tc.tile_pool(name="accs", bufs=1))
        copies = ctx.enter_context(tc.tile_pool(name="copies", bufs=6))
        scratch = ctx.enter_context(tc.tile_pool(name="scratch", bufs=3))

        ax_sb = const.tile([KA, V], mybir.dt.bfloat16)
        ay_sb = const.tile([KA, V], mybir.dt.bfloat16)
        idh_sb = const.tile([P, P], F16)
        warmsrc = const.tile([1, 1], F32)
        warm = const.tile([1, 1], F32)
        nc.vector.memset(warmsrc[:], 1.0)
        nc.scalar.activation(warm[:], warmsrc[:], mybir.ActivationFunctionType.Sqrt)
        CH = V // 2
        for c in range(2):
            nc.sync.dma_start(ax_sb[:, c * CH : (c + 1) * CH], ax_d[:, c * CH : (c + 1) * CH])
            nc.sync.dma_start(ay_sb[:, c * CH : (c + 1) * CH], ay_d[:, c * CH : (c + 1) * CH])
        # identh is consumed only by the epilogue transposes ~160us later;
        # keep it off the critical path behind the ax/ay chunks
        nc.sync.dma_start(idh_sb[:], idh_d[:])

        cacc4 = accs.tile([P, V], F16, name="cacc4")
        fold4 = accs.tile([P, RB * (V // 16)], F16, name="fold4")
        mins = accs.tile([P, 2 * MB], F32, name="mins")
        rowmin = mins[:, :MB]
        colmin = mins[:, MB:]

        with tc.tile_pool(name="psum", bufs=4, space="PSUM") as psum:
            for m in range(MB):
                lhsT = ax_sb[:, m * P : (m + 1) * P]
                ct = copies.tile([P, V], F16, name="c4k", tag="c4k")
                for g in range(NG):
                    pst = psum.tile([P, GRP], F32, name=f"ps{g}", tag="ps")
                    for j in range(GRP // NMM):
                        c0 = g * GRP + j * NMM
                        nc.tensor.matmul(
                            pst[:, j * NMM : (j + 1) * NMM],
                            lhsT,
                            ay_sb[:, c0 : c0 + NMM],
                            start=True,
                            stop=True,
                        )
                    nc.scalar.activation(
                        ct[:, g * GRP : (g + 1) * GRP], pst[:],
                        mybir.ActivationFunctionType.Relu, scale=SCL,
                    )

                # col-direction running min (one fp16 2x TT over [P, V])
                if m == 0:
                    nc.vector.tensor_copy(cacc4[:], ct[:])
                else:
                    nc.vector.tensor_tensor(cacc4[:], ct[:], cacc4[:], mn)

                # row-direction fold tree: 4096 -> 2048 -> 1024 -> 512 -> 256
                H = V // 2
                sc = scratch.tile([P, H], F16, name="sc", tag="sc")
                nc.vector.tensor_tensor(sc[:], ct[:, :H], ct[:, H:], mn)
                nc.vector.tensor_tensor(
                    sc[:, : H // 2], sc[:, : H // 2], sc[:, H // 2 :], mn
                )
                nc.vector.tensor_tensor(
                    sc[:, : H // 4], sc[:, : H // 4],
                    sc[:, H // 4 : H // 2], mn,
                )
                r = m % RB
                nc.vector.tensor_tensor(
                    fold4[:, r * (H // 8) : (r + 1) * (H // 8)],
                    sc[:, : H // 8],
                    sc[:, H // 8 : H // 4],
                    mn,
                )
                if r == RB - 1:
                    nc.vector.tensor_reduce(
                        rowmin[:, m - RB + 1 : m + 1],
                        fold4[:].rearrange("p (a b) -> p a b", a=RB),
                        axis=X,
                        op=mn,
                    )

        # Epilogue: transpose col accumulators -> per-w mins.
        with tc.tile_pool(name="psum_ep", bufs=4, space="PSUM") as psum_ep:
            for q in range(8):
                tp = psum_ep.tile([P, 4 * P], F16, name="tp", tag="tp")
                for k in range(4):
                    b = q * 4 + k
                    nc.tensor.transpose(
                        tp[:, k * P : (k + 1) * P],
                        cacc4[:, b * P : (b + 1) * P],
                        idh_sb[:],
                    )
                nc.vector.tensor_reduce(
                    colmin[:, q * 4 : q * 4 + 4],
                    tp[:].rearrange("p (a b) -> p a b", a=4),
                    axis=X,
                    op=mn,
                )

            # sqrt + fused free-dim sum (copies were relu-clamped already)
            stot = accs.tile([P, 1], F32, name="stot")
            nc.scalar.activation(
                mins[:], mins[:], mybir.ActivationFunctionType.Sqrt,
                accum_out=stot[:],
            )
            nc.sync.dma_start(loss_d[:], stot[:])

    nc.finalize()
    return nc


def _split3(v):
    """3-way bf16 split: v ~= h + m + l with residual ~2^-27 |v|."""
    f32 = np.float32
    h = v.astype(BF16)
    m = (v - h.astype(f32)).astype(BF16)
    l = (v - h.astype(f32) - m.astype(f32)).astype(BF16)
    return h, m, l


def _augment(x, y):
    """x, y: (V, 3) fp32 -> AX, AY [24, V] bf16 3-way-split gram operands.

    sq = x2 + y2 + x.(-2y); products kept: hh, hm, mh, hl, lh, mm
    (magnitude >= ~2^-16); x2/y2 carried as 3 bf16 rows each.
    """
    f32 = np.float32
    yy = (-2.0 * y).astype(f32)
    xh, xm, xl = _split3(x)
    yh, ym, yl = _split3(yy)
    x2 = np.einsum("vc,vc->v", x.astype(np.float64), x.astype(np.float64)).astype(f32)
    y2 = np.einsum("vc,vc->v", y.astype(np.float64), y.astype(np.float64)).astype(f32)
    x2h, x2m, x2l = _split3(x2)
    y2h, y2m, y2l = _split3(y2)
    one = np.ones(V, dtype=BF16)

    def cols(a):
        return [a[:, 0], a[:, 1], a[:, 2]]

    ax = np.stack(
        cols(xh) + cols(xh) + cols(xm) + cols(xh) + cols(xl) + cols(xm)
        + [x2h, x2m, x2l, one, one, one]
    )
    ay = np.stack(
        cols(yh) + cols(ym) + cols(yh) + cols(yl) + cols(yh) + cols(ym)
        + [one, one, one, y2h, y2m, y2l]
    )
    return ax, ay


def kernel(x, y):
    x = np.asarray(x, dtype=np.float32)
    y = np.asarray(y, dtype=np.float32)
    n = x.shape[0]
    assert x.shape == (n, V, 3) and y.shape == (n, V, 3) and n == 8

    if "nc" not in _cache:
        _cache["nc"] = _build_nc()
    nc = _cache["nc"]

    identh = np.eye(P, dtype=np.float16)
    in_maps = []
    for i in range(n):
        ax, ay = _augment(x[i], y[i])
        in_maps.append({"ax": ax, "ay": ay, "identh": identh})

    res = run_bass_kernel_spmd(
        nc, in_maps, list(range(n)), trace=_cache.get("trace", False)
    )
    _cache["last"] = res
    scale = 1.0 / (V * float(np.sqrt(SCL)))
    vals = [
        np.asarray(res.results[i]["loss"], dtype=np.float64).sum() * scale
        for i in range(n)
    ]
    return np.asarray(np.mean(vals), dtype=np.float32)

